# revision 5
# baseline (speedup 1.0000x reference)
"""Trainium2 Bass kernel for pointer-generator coverage attention.

reference:
  enc_feat = einsum('btn,mn->btm', h, W_h)
  dec_fea  = s_t_hat @ W_dec.T + b_dec
  att      = enc_feat + dec_fea[:,None,:] + coverage[:,:,None]*W_c
  e        = tanh(att);  scores = e @ v
  attn     = renorm(softmax(scores) * mask)
  c_t      = einsum('bt,btn->bn', attn, h)
  coverage_next = coverage + attn

Sharding: data-parallel over batch (32 -> 4 per core x 8 cores), weights
replicated. All matmuls in fp32r (full PE rate, ~1e-4 rel err). Single
program; SPMD over 8 cores via bass2jax/PJRT under axon.
"""
import sys
sys.path.insert(0, '/opt/trn_rl_repo')

import numpy as np
import concourse.bass as bass
import concourse.tile as tile
from concourse import bacc, mybir
from concourse.bass_utils import run_bass_kernel_spmd

P = 128
B, T, N = 32, 2048, 1024
NCORES = 8
NB = B // NCORES          # batches per core = 4
NSUP = T // 512           # 4 supertiles of 512 tokens
KC = N // P               # 8 contraction chunks
MC = N // P               # 8 output-feature chunks

f32 = mybir.dt.float32
f32r = mybir.dt.float32r
AF = mybir.ActivationFunctionType
ALU = mybir.AluOpType


def build_nc(repeat=1):
    nc = bacc.Bacc("TRN2", target_bir_lowering=False, debug=False)

    h_d = nc.dram_tensor("h", [NB * T, N], f32r, kind="ExternalInput").ap()
    s_d = nc.dram_tensor("s_t", [NB, N], f32r, kind="ExternalInput").ap()
    mask_d = nc.dram_tensor("mask", [NB, T], f32, kind="ExternalInput").ap()
    cov_d = nc.dram_tensor("cov", [NB, T], f32, kind="ExternalInput").ap()
    wht_d = nc.dram_tensor("w_ht", [N, N], f32r, kind="ExternalInput").ap()
    wdect_d = nc.dram_tensor("w_dect", [N, N], f32r, kind="ExternalInput").ap()
    bdec_d = nc.dram_tensor("b_dec", [1, N], f32, kind="ExternalInput").ap()
    wc_d = nc.dram_tensor("w_c", [N], f32, kind="ExternalInput").ap()
    v_d = nc.dram_tensor("v", [N], f32r, kind="ExternalInput").ap()
    ident_d = nc.dram_tensor("ident", [P, P], f32r, kind="ExternalInput").ap()

    ct_d = nc.dram_tensor("c_t", [NB, N], f32, kind="ExternalOutput").ap()
    attn_d = nc.dram_tensor("attn", [NB, T], f32, kind="ExternalOutput").ap()
    covn_d = nc.dram_tensor("covnext", [NB, T], f32, kind="ExternalOutput").ap()
    attnscr_d = nc.dram_tensor("attn_scr", [NB, T], f32).ap()  # internal scratch
    decscr_d = nc.dram_tensor("dec_scr", [NB, N], f32).ap()     # internal scratch

    from contextlib import ExitStack
    with tile.TileContext(nc) as tc, ExitStack() as ctx:
            ep = ctx.enter_context
            consts = ep(tc.tile_pool(name="consts", bufs=1))
            pstp = ep(tc.tile_pool(name="pst", bufs=2, space="PSUM"))
            psep = ep(tc.tile_pool(name="pse", bufs=2, space="PSUM"))
            pssp = ep(tc.tile_pool(name="pss", bufs=2, space="PSUM"))
            psup = ep(tc.tile_pool(name="psu", bufs=1, space="PSUM"))
            # ---- constants resident in SBUF ----
            ident = consts.tile([P, P], f32r)
            nc.sync.dma_start(ident[:], ident_d)
            wt = consts.tile([P, KC, N], f32r)                    # W_h^T  [n, m]
            nc.sync.dma_start(wt[:], wht_d.rearrange("(kc p) m -> p kc m", p=P))
            v_col = consts.tile([P, MC], f32r)
            nc.sync.dma_start(v_col[:], v_d.rearrange("(c p) -> p c", p=P))
            wc_col = consts.tile([P, MC], f32)
            nc.sync.dma_start(wc_col[:], wc_d.rearrange("(c p) -> p c", p=P))

            # ---- dec_fea = s_t_hat @ W_dec.T + b_dec  (tiny, on PE) ----
            dec_cols = consts.tile([P, NB, MC], f32)
            with tc.tile_pool(name="wdec", bufs=2) as wdp:
                sT = wdp.tile([P, NB, KC], f32r, tag="sT")
                nc.sync.dma_start(sT[:], s_d.rearrange("b (kc p) -> p b kc", p=P))
                dec_ps = psup.tile([NB, N], f32, tag="u")
                for half in range(2):
                    wdec = wdp.tile([P, KC, 512], f32r, tag="wdec")
                    nc.sync.dma_start(
                        wdec[:], wdect_d[:, half * 512:(half + 1) * 512]
                        .rearrange("(kc p) m -> p kc m", p=P))
                    for k in range(KC):
                        nc.tensor.matmul(dec_ps[:, half * 512:(half + 1) * 512],
                                         sT[:, :, k],
                                         wdec[:, k, :],
                                         start=(k == 0), stop=(k == KC - 1))
                bdec_sb = wdp.tile([NB, N], f32)
                nc.sync.dma_start(bdec_sb[:], bdec_d.to_broadcast((NB, N)))
                dec_sb = wdp.tile([NB, N], f32)
                nc.vector.tensor_add(dec_sb[:], dec_ps[:], bdec_sb[:])
                # columnize via DRAM bounce: dec_cols[p, b, c] = dec[b, c*128+p]
                nc.sync.dma_start(decscr_d[:, :], dec_sb[:])
                nc.sync.dma_start(dec_cols[:],
                                  decscr_d.rearrange("b (c p) -> p b c", p=P))

            h1p = ep(tc.tile_pool(name="h1", bufs=2))
            h2p = h1p
            hTp = ep(tc.tile_pool(name="hT", bufs=10))
            covbp = ep(tc.tile_pool(name="covb", bufs=2))
            eTp = ep(tc.tile_pool(name="eT", bufs=2))
            scoresp = ep(tc.tile_pool(name="scores", bufs=2))
            rowsp = ep(tc.tile_pool(name="rows", bufs=4))
            smallp = ep(tc.tile_pool(name="smalls", bufs=8))
            acolp = ep(tc.tile_pool(name="acol", bufs=2))
            ctp = ep(tc.tile_pool(name="ctrow", bufs=2))
            del ep

            for _rep in range(repeat):
                for b in range(NB):
                    scores_sb = scoresp.tile([1, T], f32)
                    # ================= pass 1 =================
                    for sup in range(NSUP):
                        row0 = b * T + sup * 512
                        hs = h1p.tile([P, 4, N], f32r)
                        nc.sync.dma_start(
                            hs[:], h_d[row0:row0 + 512, :]
                            .rearrange("(j p) n -> p j n", p=P))
                        # transpose h -> hT chunks [n-part, t]
                        hT = []
                        for k in range(KC):
                            pk = pstp.tile([P, 512], f32r)
                            for j in range(4):
                                nc.tensor.transpose(
                                    pk[:, j * P:(j + 1) * P],
                                    hs[:, j, k * P:(k + 1) * P], ident[:])
                            hk = hTp.tile([P, 512], f32r, tag="hT")
                            nc.scalar.copy(hk[:], pk[:])
                            hT.append(hk)
                        cb = covbp.tile([P, 512], f32)
                        nc.sync.dma_start(
                            cb[:], cov_d[b:b + 1, sup * 512:(sup + 1) * 512]
                            .to_broadcast((P, 512)))
                        score_ps = pssp.tile([1, 512], f32)
                        for m in range(MC):
                            enc_ps = psep.tile([P, 512], f32)
                            for k in range(KC):
                                nc.tensor.matmul(
                                    enc_ps[:], wt[:, k, m * P:(m + 1) * P],
                                    hT[k][:], start=(k == 0), stop=(k == KC - 1))
                            nc.vector.scalar_tensor_tensor(
                                out=enc_ps[:], in0=cb[:], scalar=wc_col[:, m:m + 1],
                                in1=enc_ps[:], op0=ALU.mult, op1=ALU.add)
                            eT = eTp.tile([P, 512], f32r)
                            nc.scalar.activation(
                                eT[:], enc_ps[:], AF.Tanh,
                                bias=dec_cols[:, b, m:m + 1])
                            nc.tensor.matmul(score_ps[:], v_col[:, m:m + 1],
                                             eT[:], start=(m == 0), stop=(m == MC - 1))
                        nc.vector.tensor_copy(
                            scores_sb[:, sup * 512:(sup + 1) * 512], score_ps[:])

                    # ================= softmax (rows, tiny) =================
                    rmax = smallp.tile([1, 1], f32)
                    nc.vector.tensor_reduce(rmax[:], scores_sb[:],
                                            mybir.AxisListType.X, ALU.max)
                    nmax = smallp.tile([1, 1], f32)
                    nc.vector.tensor_scalar_mul(nmax[:], rmax[:], -1.0)
                    p_row = rowsp.tile([1, T], f32, tag="rows")
                    nc.scalar.activation(p_row[:], scores_sb[:], AF.Exp,
                                         bias=nmax[:])
                    mask_row = rowsp.tile([1, T], f32, tag="rows")
                    nc.sync.dma_start(mask_row[:], mask_d[b:b + 1, :])
                    pm_row = p_row
                    nc.vector.tensor_mul(pm_row[:], p_row[:], mask_row[:])
                    zsum = smallp.tile([1, 1], f32)
                    nc.vector.tensor_reduce(zsum[:], pm_row[:],
                                            mybir.AxisListType.X, ALU.add)
                    invz = smallp.tile([1, 1], f32)
                    nc.vector.reciprocal(invz[:], zsum[:])
                    attn_row = pm_row
                    nc.vector.tensor_scalar_mul(attn_row[:], pm_row[:], invz[:])
                    nc.sync.dma_start(attn_d[b:b + 1, :], attn_row[:])
                    nc.sync.dma_start(attnscr_d[b:b + 1, :], attn_row[:])
                    cov_row = rowsp.tile([1, T], f32, tag="rows")
                    nc.sync.dma_start(cov_row[:], cov_d[b:b + 1, :])
                    covn_row = cov_row
                    nc.vector.tensor_add(covn_row[:], cov_row[:], attn_row[:])
                    nc.sync.dma_start(covn_d[b:b + 1, :], covn_row[:])
                    # attn as fp32r columns for the c_t matmul
                    acol = acolp.tile([P, T // P], f32r)
                    nc.gpsimd.dma_start(
                        acol[:], attnscr_d[b, :].rearrange("(tt p) -> p tt", p=P))

                    # ================= pass 2: c_t =================
                    u_ps = psup.tile([NB, N], f32, tag="u")
                    for sup in range(NSUP):
                        row0 = b * T + sup * 512
                        hs2 = h2p.tile([P, 4, N], f32r)
                        nc.sync.dma_start(
                            hs2[:], h_d[row0:row0 + 512, :]
                            .rearrange("(j p) n -> p j n", p=P))
                        for j in range(4):
                            tt = sup * 4 + j
                            for half in range(2):
                                nc.tensor.matmul(
                                    u_ps[0:1, half * 512:(half + 1) * 512],
                                    acol[:, tt:tt + 1],
                                    hs2[:, j, half * 512:(half + 1) * 512],
                                    start=(tt == 0), stop=(tt == T // P - 1))
                    ct_row = ctp.tile([1, N], f32)
                    nc.vector.tensor_copy(ct_row[:], u_ps[0:1, :])
                    nc.sync.dma_start(ct_d[b:b + 1, :], ct_row[:])
    nc.compile()
    return nc


_CACHE = {}


def _get_nc(repeat=1):
    if repeat not in _CACHE:
        _CACHE[repeat] = build_nc(repeat)
    return _CACHE[repeat]


def _make_in_maps(s_t_hat, h, enc_padding_mask, coverage, W_h, W_dec, b_dec, W_c, v):
    w_ht = np.ascontiguousarray(W_h.T).astype(np.float32)
    w_dect = np.ascontiguousarray(W_dec.T).astype(np.float32)
    ident = np.eye(P, dtype=np.float32)
    in_maps = []
    for c in range(NCORES):
        sl = slice(c * NB, (c + 1) * NB)
        in_maps.append({
            "h": np.ascontiguousarray(h[sl]).reshape(NB * T, N).astype(np.float32),
            "s_t": np.ascontiguousarray(s_t_hat[sl]).astype(np.float32),
            "mask": np.ascontiguousarray(enc_padding_mask[sl]).astype(np.float32),
            "cov": np.ascontiguousarray(coverage[sl]).astype(np.float32),
            "w_ht": w_ht, "w_dect": w_dect,
            "b_dec": b_dec.reshape(1, N).astype(np.float32),
            "w_c": W_c.astype(np.float32), "v": v.astype(np.float32),
            "ident": ident,
        })
    return in_maps


def kernel(s_t_hat, h, enc_padding_mask, coverage, W_h, W_dec, b_dec, W_c, v):
    s_t_hat, h = np.asarray(s_t_hat), np.asarray(h)
    enc_padding_mask, coverage = np.asarray(enc_padding_mask), np.asarray(coverage)
    W_h, W_dec = np.asarray(W_h), np.asarray(W_dec)
    b_dec, W_c, v = np.asarray(b_dec), np.asarray(W_c), np.asarray(v)

    nc = _get_nc(1)
    in_maps = _make_in_maps(s_t_hat, h, enc_padding_mask, coverage,
                            W_h, W_dec, b_dec, W_c, v)
    res = run_bass_kernel_spmd(nc, in_maps, core_ids=list(range(NCORES)))
    c_t = np.concatenate([r["c_t"] for r in res.results], axis=0)
    attn = np.concatenate([r["attn"] for r in res.results], axis=0)
    covn = np.concatenate([r["covnext"] for r in res.results], axis=0)
    return c_t, attn, covn


# revision 15
# speedup vs baseline: 6.7125x; 6.7125x over previous
"""Trainium2 Bass kernel for pointer-generator coverage attention.

reference math:
  enc_feat = einsum('btn,mn->btm', h, W_h)
  dec_fea  = s_t_hat @ W_dec.T + b_dec
  att      = enc_feat + dec_fea[:,None,:] + coverage[:,:,None]*W_c
  e        = tanh(att);  scores = e @ v
  attn     = renorm(softmax(scores) * mask)
  c_t      = einsum('bt,btn->bn', attn, h)
  coverage_next = coverage + attn

Sharding: data-parallel over batch (32 -> 4 per core x 8 cores), weights
replicated. All matmuls fp32r (full PE rate, ~1e-4 rel err).

Single pass over h (flash-style): per 512-token supertile we compute
scores, exponentiate UNNORMALIZED (scores are bounded by ||v||_1, so
exp() cannot overflow fp32 and no running-max rescale is needed),
and immediately accumulate U = sum_t p_t*mask_t*h_t on the PE into PSUM
while h is still resident in SBUF. At the end of each batch row:
Z = sum(p*mask), attn = p*mask/Z, c_t = U/Z.
"""
import sys
sys.path.insert(0, '/opt/trn_rl_repo')

import numpy as np
import concourse.bass as bass
import concourse.tile as tile
from concourse import bacc, mybir
from concourse.bass_utils import run_bass_kernel_spmd

P = 128
B, T, N = 32, 2048, 1024
NCORES = 8
NB = B // NCORES          # batches per core = 4
NSUP = T // 512           # 4 supertiles of 512 tokens
KC = N // P               # 8 contraction chunks
MC = N // P               # 8 output-feature chunks

f32 = mybir.dt.float32
f32r = mybir.dt.float32r
AF = mybir.ActivationFunctionType
ALU = mybir.AluOpType


def build_nc(repeat=1, trace_sim=False, loop_repeat=0, mc_run=MC,
             do_transpose=True, pst_bufs=3, pse_bufs=2, h1_bufs=3, hT_bufs=10):
    nc = bacc.Bacc("TRN2", target_bir_lowering=False, debug=False)

    h_d = nc.dram_tensor("h", [NB * T, N], f32r, kind="ExternalInput").ap()
    s_d = nc.dram_tensor("s_t", [NB, N], f32r, kind="ExternalInput").ap()
    mask_d = nc.dram_tensor("mask", [NB, T], f32, kind="ExternalInput").ap()
    cov_d = nc.dram_tensor("cov", [NB, T], f32, kind="ExternalInput").ap()
    wht_d = nc.dram_tensor("w_ht", [N, N], f32r, kind="ExternalInput").ap()
    wdect_d = nc.dram_tensor("w_dect", [N, N], f32r, kind="ExternalInput").ap()
    bdec_d = nc.dram_tensor("b_dec", [1, N], f32, kind="ExternalInput").ap()
    wc_d = nc.dram_tensor("w_c", [N], f32, kind="ExternalInput").ap()
    v_d = nc.dram_tensor("v", [N], f32r, kind="ExternalInput").ap()
    ident_d = nc.dram_tensor("ident", [P, P], f32r, kind="ExternalInput").ap()
    ident4_d = nc.dram_tensor("ident4", [P, 4, 512], f32r, kind="ExternalInput").ap()

    ct_d = nc.dram_tensor("c_t", [NB, N], f32, kind="ExternalOutput").ap()
    attn_d = nc.dram_tensor("attn", [NB, T], f32, kind="ExternalOutput").ap()
    covn_d = nc.dram_tensor("covnext", [NB, T], f32, kind="ExternalOutput").ap()
    pmscr_d = nc.dram_tensor("pm_scr", [NB, T], f32).ap()   # internal scratch
    decscr_d = nc.dram_tensor("dec_scr", [NB, N], f32).ap()  # internal scratch

    from contextlib import ExitStack, nullcontext
    with tile.TileContext(nc, trace_sim=trace_sim) as tc, ExitStack() as ctx:
        ep = ctx.enter_context
        consts = ep(tc.tile_pool(name="consts", bufs=1))
        pstp = ep(tc.tile_pool(name="pst", bufs=pst_bufs, space="PSUM"))
        psep = ep(tc.tile_pool(name="pse", bufs=pse_bufs, space="PSUM"))
        pssp = ep(tc.tile_pool(name="pss", bufs=1, space="PSUM"))
        psup = ep(tc.tile_pool(name="psu", bufs=1, space="PSUM"))

        # ---- constants resident in SBUF ----
        ident = consts.tile([P, P], f32r)
        nc.sync.dma_start(ident[:], ident_d)
        ident4 = consts.tile([P, 4, 512], f32r)
        nc.sync.dma_start(ident4[:], ident4_d)
        wt = consts.tile([P, KC, N], f32r)                    # W_h^T  [n, m]
        nc.sync.dma_start(wt[:], wht_d.rearrange("(kc p) m -> p kc m", p=P))
        v_col = consts.tile([P, MC], f32r)
        nc.sync.dma_start(v_col[:], v_d.rearrange("(c p) -> p c", p=P))
        wc_col = consts.tile([P, MC], f32)
        nc.sync.dma_start(wc_col[:], wc_d.rearrange("(c p) -> p c", p=P))

        # ---- dec_fea = s_t_hat @ W_dec.T + b_dec  (tiny, on PE) ----
        dec_cols = consts.tile([P, NB, MC], f32)
        with tc.tile_pool(name="wdec", bufs=2) as wdp:
            sT = wdp.tile([P, NB, KC], f32r, tag="sT")
            nc.sync.dma_start(sT[:], s_d.rearrange("b (kc p) -> p b kc", p=P))
            dec_ps = psup.tile([NB, N], f32, tag="u")
            for half in range(2):
                wdec = wdp.tile([P, KC, 512], f32r, tag="wdec")
                nc.sync.dma_start(
                    wdec[:], wdect_d[:, half * 512:(half + 1) * 512]
                    .rearrange("(kc p) m -> p kc m", p=P))
                for k in range(KC):
                    nc.tensor.matmul(dec_ps[:, half * 512:(half + 1) * 512],
                                     sT[:, :, k], wdec[:, k, :],
                                     start=(k == 0), stop=(k == KC - 1))
            bdec_sb = wdp.tile([NB, N], f32)
            nc.sync.dma_start(bdec_sb[:], bdec_d.to_broadcast((NB, N)))
            dec_sb = wdp.tile([NB, N], f32)
            nc.vector.tensor_add(dec_sb[:], dec_ps[:], bdec_sb[:])
            nc.sync.dma_start(decscr_d[:, :], dec_sb[:])
            nc.sync.dma_start(dec_cols[:],
                              decscr_d.rearrange("b (c p) -> p b c", p=P))

        # ---- streaming pools ----
        h1p = ep(tc.tile_pool(name="h1", bufs=h1_bufs))
        hTp = ep(tc.tile_pool(name="hT", bufs=hT_bufs))
        covbp = ep(tc.tile_pool(name="covb", bufs=2))
        eTp = ep(tc.tile_pool(name="eT", bufs=2))
        rowsp = ep(tc.tile_pool(name="rows", bufs=2))
        smallp = ep(tc.tile_pool(name="smalls", bufs=8))
        acolp = ep(tc.tile_pool(name="acol", bufs=3))
        ctp = ep(tc.tile_pool(name="ctrow", bufs=1))
        del ep

        loop_cm = tc.For_i(0, loop_repeat, 1) if loop_repeat else nullcontext()
        with loop_cm:
          for _rep in range(repeat):
            for b in range(NB):
                mask_row = rowsp.tile([1, T], f32, tag="mask")
                nc.sync.dma_start(mask_row[:], mask_d[b:b + 1, :])
                pm_row = rowsp.tile([1, T], f32, tag="pm")
                u_ps = psup.tile([NB, N], f32, tag="u")
                for sup in range(NSUP):
                    sl = slice(sup * 512, (sup + 1) * 512)
                    row0 = b * T + sup * 512
                    hs = h1p.tile([P, 4, N], f32r)
                    nc.sync.dma_start(
                        hs[:], h_d[row0:row0 + 512, :]
                        .rearrange("(j p) n -> p j n", p=P))
                    # transpose h -> hT chunks [n-part, t]
                    hT = []
                    if do_transpose == "mm4":
                        for k in range(KC):
                            pk = pstp.tile([P, 512], f32, tag="pk")
                            for j in range(4):
                                nc.tensor.matmul(
                                    pk[:], hs[:, j, k * P:(k + 1) * P],
                                    ident4[:, j, :],
                                    start=(j == 0), stop=(j == 3))
                            hk = hTp.tile([P, 512], f32r, tag="hT")
                            nc.scalar.copy(hk[:], pk[:])
                            hT.append(hk)
                    elif do_transpose:
                        for k in range(KC):
                            pk = pstp.tile([P, 512], f32r)
                            for j in range(4):
                                nc.tensor.transpose(
                                    pk[:, j * P:(j + 1) * P],
                                    hs[:, j, k * P:(k + 1) * P], ident[:])
                            hk = hTp.tile([P, 512], f32r, tag="hT")
                            nc.scalar.copy(hk[:], pk[:])
                            hT.append(hk)
                    else:
                        hT = [hs[:, k % 4, 0:512] for k in range(KC)]
                    cb = covbp.tile([P, 512], f32)
                    nc.sync.dma_start(
                        cb[:], cov_d[b:b + 1, sl].to_broadcast((P, 512)))
                    score_ps = pssp.tile([1, 512], f32)
                    for m in range(mc_run):
                        enc_ps = psep.tile([P, 512], f32)
                        for k in range(KC):
                            nc.tensor.matmul(
                                enc_ps[:], wt[:, k, m * P:(m + 1) * P],
                                hT[k][:], start=(k == 0), stop=(k == KC - 1))
                        nc.vector.scalar_tensor_tensor(
                            out=enc_ps[:], in0=cb[:], scalar=wc_col[:, m:m + 1],
                            in1=enc_ps[:], op0=ALU.mult, op1=ALU.add)
                        eT = eTp.tile([P, 512], f32r)
                        nc.scalar.activation(
                            eT[:], enc_ps[:], AF.Tanh,
                            bias=dec_cols[:, b, m:m + 1])
                        nc.tensor.matmul(score_ps[:], v_col[:, m:m + 1],
                                         eT[:], start=(m == 0),
                                         stop=(m == mc_run - 1))
                    # p = exp(scores) (unnormalized; bounded), masked
                    nc.scalar.activation(pm_row[:, sl], score_ps[:], AF.Exp)
                    nc.vector.tensor_mul(pm_row[:, sl], pm_row[:, sl],
                                         mask_row[:, sl])
                    nc.sync.dma_start(pmscr_d[b:b + 1, sl], pm_row[:, sl])
                    acol = acolp.tile([P, 4], f32r)
                    nc.gpsimd.dma_start(
                        acol[:], pmscr_d[b, sl].rearrange("(tt p) -> p tt", p=P))
                    # U += p_t * h_t  (accumulate across all sups of this b)
                    for j in range(4):
                        for half in range(2):
                            nc.tensor.matmul(
                                u_ps[0:1, half * 512:(half + 1) * 512],
                                acol[:, j:j + 1],
                                hs[:, j, half * 512:(half + 1) * 512],
                                start=(sup == 0 and j == 0),
                                stop=(sup == NSUP - 1 and j == 3))

                # ---- finalize batch row ----
                zsum = smallp.tile([1, 1], f32)
                nc.vector.tensor_reduce(zsum[:], pm_row[:],
                                        mybir.AxisListType.X, ALU.add)
                invz = smallp.tile([1, 1], f32)
                nc.vector.reciprocal(invz[:], zsum[:])
                attn_row = rowsp.tile([1, T], f32, tag="attn")
                nc.vector.tensor_scalar_mul(attn_row[:], pm_row[:], invz[:])
                nc.sync.dma_start(attn_d[b:b + 1, :], attn_row[:])
                cov_row = rowsp.tile([1, T], f32, tag="covr")
                nc.sync.dma_start(cov_row[:], cov_d[b:b + 1, :])
                nc.vector.tensor_add(cov_row[:], cov_row[:], attn_row[:])
                nc.sync.dma_start(covn_d[b:b + 1, :], cov_row[:])
                ct_row = ctp.tile([1, N], f32)
                nc.vector.tensor_scalar_mul(ct_row[:], u_ps[0:1, :], invz[:])
                nc.sync.dma_start(ct_d[b:b + 1, :], ct_row[:])
    nc.compile()
    return nc


_CACHE = {}


def _get_nc(repeat=1):
    if repeat not in _CACHE:
        _CACHE[repeat] = build_nc(repeat)
    return _CACHE[repeat]


def _make_in_maps(s_t_hat, h, enc_padding_mask, coverage, W_h, W_dec, b_dec, W_c, v):
    w_ht = np.ascontiguousarray(W_h.T).astype(np.float32)
    w_dect = np.ascontiguousarray(W_dec.T).astype(np.float32)
    ident = np.eye(P, dtype=np.float32)
    ident4 = np.zeros((P, 4, 512), np.float32)
    for j in range(4):
        ident4[:, j, j * P:(j + 1) * P] = np.eye(P)
    in_maps = []
    for c in range(NCORES):
        sl = slice(c * NB, (c + 1) * NB)
        in_maps.append({
            "h": np.ascontiguousarray(h[sl]).reshape(NB * T, N).astype(np.float32),
            "s_t": np.ascontiguousarray(s_t_hat[sl]).astype(np.float32),
            "mask": np.ascontiguousarray(enc_padding_mask[sl]).astype(np.float32),
            "cov": np.ascontiguousarray(coverage[sl]).astype(np.float32),
            "w_ht": w_ht, "w_dect": w_dect,
            "b_dec": b_dec.reshape(1, N).astype(np.float32),
            "w_c": W_c.astype(np.float32), "v": v.astype(np.float32),
            "ident": ident, "ident4": ident4,
        })
    return in_maps


def kernel(s_t_hat, h, enc_padding_mask, coverage, W_h, W_dec, b_dec, W_c, v):
    s_t_hat, h = np.asarray(s_t_hat), np.asarray(h)
    enc_padding_mask, coverage = np.asarray(enc_padding_mask), np.asarray(coverage)
    W_h, W_dec = np.asarray(W_h), np.asarray(W_dec)
    b_dec, W_c, v = np.asarray(b_dec), np.asarray(W_c), np.asarray(v)

    nc = _get_nc(1)
    in_maps = _make_in_maps(s_t_hat, h, enc_padding_mask, coverage,
                            W_h, W_dec, b_dec, W_c, v)
    res = run_bass_kernel_spmd(nc, in_maps, core_ids=list(range(NCORES)))
    c_t = np.concatenate([r["c_t"] for r in res.results], axis=0)
    attn = np.concatenate([r["attn"] for r in res.results], axis=0)
    covn = np.concatenate([r["covnext"] for r in res.results], axis=0)
    return c_t, attn, covn
